# revision 3
# baseline (speedup 1.0000x reference)
"""Trainium2 Bass kernel for nn_DeformableBlock (deformable conv v1 block).

Contract: kernel(**inputs) takes FULL unsharded inputs, returns FULL output.
Sharding: data-parallel over batch (B=8 -> 8 NeuronCores, 1 batch each),
weights replicated.

Per-core algorithm (one batch, Cin=128, Cout=256, H=W=64), v5:
  1. offset conv (3x3, pad 1) as 9 shifted bf16 matmuls -> offset [18, 4096]
  2. PE-transpose offsets to pixel-major; DVE computes, per (pixel, k):
     a 2x2 anchor (ay, bx) = clamp(floor-sample corner) and 4 tap weights
     w4 with clamp-swap logic (invalid taps get weight 0, data stays
     in-bounds), plus one gather index idx = ay*64 + bx in [0, 4030].
  3. idx is wrapped to the dma_gather layout (i%16 partitions) entirely
     on-chip with two rounds of PE transposes (no DRAM round trip, no
     xbar-fallback descriptor storm).
  4. Z table in DRAM: Z[j] = (xT[j], xT[j+64], xT[j+1], xT[j+65]) bf16 --
     1 KB per pixel holding the full 2x2 bilinear patch, so ONE gather
     descriptor per (k, pixel) fetches all 4 taps (halves Q7 SWDGE time).
  5. per (k, half): dma_gather -> G [128 jp, 16 jt, 512]; DVE tap weighting
     with per-partition scalars (ts + 3 stt) -> val [128 jp, 16, 128] bf16;
     one batched xbar DMA-transpose -> valT [128 c, 16, 128 jp] = channel-
     major j-natural.
  6. main conv: PSUM-resident accumulation over k: out[oh, jc] += W_k @ valT
     (8 PSUM banks = 2 oh x 4 jc chunks of [128, 512] f32 per half).
  7. bias + ReLU on ScalarE, DMA out [256, 4096] f32
"""
import os
import sys
import numpy as np

try:
    import concourse.bass as bass
except ImportError:  # pragma: no cover
    sys.path.insert(0, '/opt/trn_rl_repo')
    import concourse.bass as bass
import concourse.bacc as bacc

import concourse.mybir as mybir
import concourse.tile as tile
from concourse.bass_utils import run_bass_kernel_spmd

F32 = mybir.dt.float32
BF16 = mybir.dt.bfloat16
I32 = mybir.dt.int32
I16 = mybir.dt.int16
ALU = mybir.AluOpType
ACTF = mybir.ActivationFunctionType

B, CIN, COUT, H, W = 8, 128, 256, 64, 64
HW = H * W          # 4096
NJT = HW // 128     # 32 pixel-major tiles
NK = 9
KY = [(-1), (-1), (-1), 0, 0, 0, 1, 1, 1]
KX = [(-1), 0, 1, (-1), 0, 1, (-1), 0, 1]
NHALF = 2
JH = HW // NHALF    # 2048 pixels per half
ZW = 4 * CIN        # 512 elements per Z row (2x2 patch, channel-major taps)

_CACHE = {}


def _split_multiwaits(nc, max_waits=1, kinds=None):
    """walrus CoreV3 codegen rejects control instructions carrying more
    than one sem-wait; split the excess into a chain of same-engine
    drains placed directly before the offender."""
    if kinds is None:
        kinds = (mybir.InstDrain,)
    n_split = 0
    for fn in nc.m.functions:
        for bb in fn.blocks:
            insts = list(bb.instructions)
            new = []
            changed = False
            for inst in insts:
                si = inst.sync_info
                if (isinstance(inst, kinds) and si is not None
                        and len(si.on_wait) > max_waits):
                    waits = list(si.on_wait)
                    pre, rest = waits[:-max_waits], waits[-max_waits:]
                    for i in range(0, len(pre), max_waits):
                        chunk = pre[i:i + max_waits]
                        d = mybir.InstDrain(
                            name=f"{inst.name}-wsplit{i}",
                            engine=inst.engine,
                            ins=[], outs=[],
                            sync_info=mybir.SyncInfo(
                                on_wait=chunk, on_update=[]),
                        )
                        new.append(d)
                        n_split += 1
                    inst.sync_info = mybir.SyncInfo(
                        on_wait=rest, on_update=list(si.on_update))
                    changed = True
                new.append(inst)
            if changed:
                bb.instructions = new
    return n_split


def _build_program(phase=3):
    nc = bacc.Bacc('TRN2', target_bir_lowering=False, debug=False,
                   enable_asserts=False, num_devices=B)

    # ---- DRAM I/O ----
    xp_d = nc.dram_tensor('xp', [CIN, 66 * 66], BF16, kind='ExternalInput')
    woffT_d = nc.dram_tensor('woffT', [9, CIN, 18], BF16, kind='ExternalInput')
    boff_d = nc.dram_tensor('boff', [18, 1], F32, kind='ExternalInput')
    wdefT_d = nc.dram_tensor('wdefT', [NK, CIN, COUT], BF16, kind='ExternalInput')
    bdef_d = nc.dram_tensor('bdef', [128, 2], F32, kind='ExternalInput')
    ident_d = nc.dram_tensor('ident', [128, 128], F32, kind='ExternalInput')
    hgk_d = nc.dram_tensor('hgk', [128, NJT, NK], F32, kind='ExternalInput')
    wgk_d = nc.dram_tensor('wgk', [128, NJT, NK], F32, kind='ExternalInput')
    y_d = nc.dram_tensor('y', [COUT, HW], F32, kind='ExternalOutput')

    # DRAM scratch: Z[j] = (xT[j], xT[j+64], xT[j+1], xT[j+65])
    z_d = nc.dram_tensor('z_scratch', [HW, ZW], BF16, kind='Internal')

    with tile.TileContext(nc) as tc:
        with tc.tile_pool(name='const', bufs=1) as cpool:
            # persistent small tensors
            wdefT = cpool.tile([CIN, NK, COUT], BF16, tag='wdefT')
            nc.sync.dma_start(
                wdefT[:], wdefT_d.ap().rearrange('k c o -> c k o'))
            bdef = cpool.tile([128, 2], F32, tag='bdef')
            nc.sync.dma_start(bdef[:], bdef_d.ap())
            # wrapped gather indices: [p, k, half, fa, fb]; per (k, half) the
            # free slice (fa, fb) is the contiguous 128-entry idx/16 dim.
            idxwr = cpool.tile([128, NK, NHALF, 16, 8], I16, tag='idxwr')
            w4 = cpool.tile([128, NJT, NK, 4], F32, tag='w4')

            # ======== phase 1: offsets, indices, weights, Z ========
            with (
                tc.tile_pool(name='p1', bufs=1) as apool,
                tc.tile_pool(name='ps_small', bufs=2, space='PSUM') as ps_small,
            ):
                xbf = apool.tile([CIN, 66 * 66], BF16, tag='xbf')
                nc.sync.dma_start(xbf[:], xp_d.ap())
                ident = apool.tile([128, 128], F32, tag='ident')
                nc.sync.dma_start(ident[:], ident_d.ap())
                identb = apool.tile([128, 128], BF16, tag='identb')
                nc.vector.tensor_copy(identb[:], ident[:])
                woffT = apool.tile([CIN, 9, 18], BF16, tag='woffT')
                nc.sync.dma_start(
                    woffT[:], woffT_d.ap().rearrange('s c o -> c s o'))
                boff = apool.tile([18, 1], F32, tag='boff')
                nc.sync.dma_start(boff[:], boff_d.ap())
                hgk = apool.tile([128, NJT, NK], F32, tag='hgk')
                nc.sync.dma_start(hgk[:], hgk_d.ap())
                wgk = apool.tile([128, NJT, NK], F32, tag='wgk')
                nc.sync.dma_start(wgk[:], wgk_d.ap())

                # ---------- offset conv: off [18, 4096] f32 ----------
                # Conv runs on the padded 66-wide grid so the streaming
                # operand is a single contiguous run; the interior is
                # extracted in the ACT epilogue.
                off_sb = apool.tile([18, HW], F32, tag='off_sb')
                chunks = [(1 + 7 * i, 7) for i in range(9)] + [(64, 1)]
                for r0, nr in chunks:
                    nfree = 66 * (nr - 1) + 64
                    ps = ps_small.tile([18, 512], F32, tag='ps', name='ps')
                    for s in range(9):
                        dh, dw = s // 3, s % 3
                        beg = r0 * 66 + 1 + (dh - 1) * 66 + (dw - 1)
                        rhs = bass.AP(
                            tensor=xbf[:].tensor,
                            offset=xbf[:].offset + beg,
                            ap=[list(xbf[:].ap[0]), [1, nfree]],
                        )
                        nc.tensor.matmul(
                            ps[:, :nfree], lhsT=woffT[:, s, :], rhs=rhs,
                            start=(s == 0), stop=(s == 8))
                    src_in = bass.AP(
                        tensor=ps[:].tensor, offset=ps[:].offset,
                        ap=[list(ps[:].ap[0]), [66, nr], [1, 64]])
                    nc.scalar.activation(
                        off_sb[:, 64 * (r0 - 1):64 * (r0 - 1 + nr)], src_in,
                        ACTF.Identity, bias=boff[:], scale=1.0)

                # ---------- transpose offsets to pixel-major ----------
                offT = apool.tile([128, NJT, 18], F32, tag='offT')
                for jt in range(NJT):
                    ps = ps_small.tile([128, 18], F32, tag='ps')
                    nc.tensor.transpose(
                        ps[:], off_sb[:, 128 * jt:128 * (jt + 1)],
                        ident[:18, :18])
                    nc.vector.tensor_copy(offT[:, jt, :], ps[:])

                # ---------- xT build: xTsb [128 jp, 32 jt, 128 c] ----------
                xc = apool.tile([CIN, HW], BF16, tag='xc')
                xin_all = bass.AP(
                    tensor=xbf[:].tensor,
                    offset=xbf[:].offset + 67,
                    ap=[list(xbf[:].ap[0]), [66, 64], [1, 64]])
                nc.vector.tensor_copy(xc[:], xin_all)
                xTsb = apool.tile([128, NJT, CIN], BF16, tag='xTsb')
                for jt in range(NJT):
                    ps = ps_small.tile([128, 128], BF16, tag='psb', name='ps')
                    nc.tensor.transpose(
                        ps[:], xc[:, 128 * jt:128 * (jt + 1)], identb[:])
                    nc.vector.tensor_copy(xTsb[:, jt, :], ps[:])

                # ---------- Z build (7 DMAs; row j holds the 2x2 patch) ----
                # Z[:, blk] = xT[j + d], d in (0, 64, 1, 65).  Source rows
                # come from xTsb with a partition shift; rows of Z that
                # would read past xT (j + d >= 4096) are never gathered
                # (idx <= 4030), so they are simply skipped.
                def zdst(blk, p0, p1, jt0, jt1):
                    full = z_d.ap()[:, 128 * blk:128 * (blk + 1)].rearrange(
                        '(jt jp) c -> jp jt c', jp=128)
                    return full[p0:p1, jt0:jt1, :]

                # blk 0: d=0
                nc.sync.dma_start(zdst(0, 0, 128, 0, 32), xTsb[:])
                # blk 1: d=64 -> src (jp+64, jt) / (jp-64, jt+1)
                nc.sync.dma_start(zdst(1, 0, 64, 0, 32), xTsb[64:128, :, :])
                nc.sync.dma_start(zdst(1, 64, 128, 0, 31), xTsb[0:64, 1:32, :])
                # blk 2: d=1 -> src (jp+1, jt) / (0, jt+1)
                nc.sync.dma_start(zdst(2, 0, 127, 0, 32), xTsb[1:128, :, :])
                nc.sync.dma_start(zdst(2, 127, 128, 0, 31), xTsb[0:1, 1:32, :])
                # blk 3: d=65 -> src (jp+65, jt) / (jp-63, jt+1)
                nc.sync.dma_start(zdst(3, 0, 63, 0, 32), xTsb[65:128, :, :])
                nc.sync.dma_start(zdst(3, 63, 128, 0, 31), xTsb[0:65, 1:32, :])

                # ---------- index/weight arithmetic (pixel-major) ----------
                sh = [128, NJT, NK]

                def T(tag, dt=F32):
                    return apool.tile(sh, dt, tag=tag, name=tag)

                dyx = offT[:].rearrange('p jt (k two) -> p jt k two', two=2)
                dy = dyx[:, :, :, 0]
                dx = dyx[:, :, :, 1]

                ti = apool.tile(sh, I32, tag='ti')
                fdy, fdx = T('fdy'), T('fdx')
                tmp1, tmp2, tmp3 = T('tmp1'), T('tmp2'), T('tmp3')
                # floor(dy)
                nc.vector.tensor_copy(ti[:], dy)
                nc.vector.tensor_copy(fdy[:], ti[:])
                nc.vector.tensor_tensor(tmp1[:], fdy[:], dy, ALU.is_gt)
                nc.vector.tensor_tensor(fdy[:], fdy[:], tmp1[:], ALU.subtract)
                # floor(dx)
                nc.vector.tensor_copy(ti[:], dx)
                nc.vector.tensor_copy(fdx[:], ti[:])
                nc.vector.tensor_tensor(tmp1[:], fdx[:], dx, ALU.is_gt)
                nc.vector.tensor_tensor(fdx[:], fdx[:], tmp1[:], ALU.subtract)

                ly, lx = T('ly'), T('lx')
                nc.vector.tensor_tensor(ly[:], dy, fdy[:], ALU.subtract)
                nc.vector.tensor_tensor(lx[:], dx, fdx[:], ALU.subtract)

                y0, x0 = T('y0'), T('x0')
                nc.vector.tensor_tensor(y0[:], hgk[:], fdy[:], ALU.add)
                nc.vector.tensor_tensor(x0[:], wgk[:], fdx[:], ALU.add)

                # y anchor + tap-swap terms
                ay, y01, ayp = T('ay'), T('y01'), T('ayp')
                f0, f1, f3 = T('f0'), T('f1'), T('f3')
                nc.vector.tensor_scalar(ay[:], y0[:], 62.0, 0.0, ALU.min, ALU.max)
                nc.vector.tensor_scalar(y01[:], y0[:], 1.0, None, ALU.add)
                nc.vector.tensor_scalar(ayp[:], ay[:], 1.0, None, ALU.add)
                nc.vector.tensor_tensor(f0[:], y0[:], ay[:], ALU.is_equal)
                nc.vector.tensor_tensor(f1[:], y01[:], ay[:], ALU.is_equal)
                nc.vector.tensor_tensor(f3[:], y0[:], ayp[:], ALU.is_equal)

                # x anchor + tap-swap terms
                bx, x01, bxp = T('bx'), T('x01'), T('bxp')
                e0, e1, e3 = T('e0'), T('e1'), T('e3')
                nc.vector.tensor_scalar(bx[:], x0[:], 62.0, 0.0, ALU.min, ALU.max)
                nc.vector.tensor_scalar(x01[:], x0[:], 1.0, None, ALU.add)
                nc.vector.tensor_scalar(bxp[:], bx[:], 1.0, None, ALU.add)
                nc.vector.tensor_tensor(e0[:], x0[:], bx[:], ALU.is_equal)
                nc.vector.tensor_tensor(e1[:], x01[:], bx[:], ALU.is_equal)
                nc.vector.tensor_tensor(e3[:], x0[:], bxp[:], ALU.is_equal)

                # wyA = (1-ly)f0 + ly*f1 ; wyB = ly*f0 + (1-ly)*f3
                wyA, wyB = T('wyA'), T('wyB')
                nc.vector.tensor_tensor(tmp1[:], ly[:], f0[:], ALU.mult)
                nc.vector.tensor_tensor(wyA[:], f0[:], tmp1[:], ALU.subtract)
                nc.vector.tensor_tensor(tmp2[:], ly[:], f1[:], ALU.mult)
                nc.vector.tensor_tensor(wyA[:], wyA[:], tmp2[:], ALU.add)
                nc.vector.tensor_tensor(tmp2[:], ly[:], f3[:], ALU.mult)
                nc.vector.tensor_tensor(tmp3[:], f3[:], tmp2[:], ALU.subtract)
                nc.vector.tensor_tensor(wyB[:], tmp1[:], tmp3[:], ALU.add)

                wxA, wxB = T('wxA'), T('wxB')
                nc.vector.tensor_tensor(tmp1[:], lx[:], e0[:], ALU.mult)
                nc.vector.tensor_tensor(wxA[:], e0[:], tmp1[:], ALU.subtract)
                nc.vector.tensor_tensor(tmp2[:], lx[:], e1[:], ALU.mult)
                nc.vector.tensor_tensor(wxA[:], wxA[:], tmp2[:], ALU.add)
                nc.vector.tensor_tensor(tmp2[:], lx[:], e3[:], ALU.mult)
                nc.vector.tensor_tensor(tmp3[:], e3[:], tmp2[:], ALU.subtract)
                nc.vector.tensor_tensor(wxB[:], tmp1[:], tmp3[:], ALU.add)

                # tap weights (Z block order: (yA,xA),(yB,xA),(yA,xB),(yB,xB))
                nc.vector.tensor_tensor(w4[:, :, :, 0], wyA[:], wxA[:], ALU.mult)
                nc.vector.tensor_tensor(w4[:, :, :, 1], wyB[:], wxA[:], ALU.mult)
                nc.vector.tensor_tensor(w4[:, :, :, 2], wyA[:], wxB[:], ALU.mult)
                nc.vector.tensor_tensor(w4[:, :, :, 3], wyB[:], wxB[:], ALU.mult)

                # gather index idx = ay*64 + bx  (exact int in f32, <= 4030)
                idxf = T('idxf')
                nc.vector.tensor_scalar(tmp1[:], ay[:], 64.0, None, ALU.mult)
                nc.vector.tensor_tensor(idxf[:], tmp1[:], bx[:], ALU.add)

                # ---------- on-chip idx wrap (PE transposes) ----------
                # Round 1: pixel-major [jp, jt] -> j-natural rows [jt, jp].
                idxT = apool.tile([NJT, NK, 128], F32, tag='idxT')
                for k in range(NK):
                    ps = ps_small.tile([NJT, 128], F32, tag='ps')
                    nc.tensor.transpose(ps[:], idxf[:, :, k], ident[:])
                    nc.vector.tensor_copy(idxT[:, k, :], ps[:])
                # Round 2: [jt, fb*16+p] -> [p, (half, fa=jtl)] per (k, fb),
                # cast to i16 on the PSUM evacuation.
                for k in range(NK):
                    for fb in range(8):
                        ps = ps_small.tile([16, NJT], F32, tag='ps16',
                                           name='ps16')
                        nc.tensor.transpose(
                            ps[:], idxT[:, k, 16 * fb:16 * (fb + 1)],
                            ident[:NJT, :NJT])
                        nc.vector.tensor_copy(
                            idxwr[0:16, k, :, :, fb],
                            ps[:].rearrange('p (h a) -> p h a', h=NHALF))
                # replicate 16 -> 128 partitions (3 doubling copies)
                for g in (16, 32, 64):
                    nc.sync.dma_start(idxwr[g:2 * g], idxwr[0:g])

            # ======== phase 2: gather + weighting + transpose + conv ========
            if phase != 1:
                _phase2(nc, tc, wdefT, bdef, idxwr, w4, z_d, y_d, phase)
            if phase != 3:
                with tc.tile_pool(name='zz', bufs=1) as zp:
                    zt = zp.tile([128, HW], F32, tag='zt')
                    nc.vector.memset(zt[:], 0)
                    for oh in range(2):
                        nc.sync.dma_start(
                            y_d.ap()[128 * oh:128 * (oh + 1), :], zt[:])

    nc.finalize()
    _split_multiwaits(nc)
    return nc


def _phase2(nc, tc, wdefT, bdef, idxwr, w4, z_d, y_d, phase=3):
    with (
        tc.tile_pool(name='gath', bufs=2) as gpool,
        tc.tile_pool(name='val', bufs=2) as vpool,
        tc.tile_pool(name='valT', bufs=3) as vtpool,
        tc.tile_pool(name='outp', bufs=2) as opool,
        tc.tile_pool(name='ps8', bufs=1, space='PSUM') as ps8,
    ):
        for half in range(NHALF):
            if phase == 3:
                psc = [ps8.tile([128, 512], F32, tag=f'psc{i}',
                                name=f'psc{i}') for i in range(8)]
            for k in range(NK):
                # ---- gather: one 1KB descriptor per pixel, all 4 taps ----
                G = gpool.tile([128, JH // 128, ZW], BF16, tag='G', name='G')
                in_ap = bass.AP(
                    tensor=z_d, offset=0,
                    ap=[[ZW, HW], [1, ZW]],
                )
                nc.gpsimd.dma_gather(
                    out_ap=G[:],
                    in_ap=in_ap,
                    idxs_ap=idxwr[:, k, half, :, :],
                    num_idxs=JH,
                    num_idxs_reg=JH,
                    elem_size=ZW,
                    elem_step=ZW,
                    transpose=False,
                    single_packet=False,
                )
                if phase == 21:
                    continue

                # ---- tap weighting (pixel-major, per-partition scalars) ----
                val = vpool.tile([128, JH // 128, CIN], BF16, tag='val',
                                 name='val')
                for jtl in range(JH // 128):
                    jt = half * (JH // 128) + jtl
                    vsl = val[:, jtl, :]
                    nc.vector.tensor_scalar(
                        vsl, G[:, jtl, 0:CIN],
                        w4[:, jt, k, 0].unsqueeze(1), None, ALU.mult)
                    for t in (1, 2, 3):
                        nc.vector.scalar_tensor_tensor(
                            vsl, G[:, jtl, t * CIN:(t + 1) * CIN],
                            w4[:, jt, k, t].unsqueeze(1), vsl,
                            ALU.mult, ALU.add)

                # ---- batched xbar transpose -> channel-major j-natural ----
                valT = vtpool.tile([128, JH // 128, 128], BF16, tag='valT',
                                   name='valT')
                nc.sync.dma_start_transpose(valT[:], val[:])

                if phase != 3:
                    continue
                # ---- main conv: accumulate over k in PSUM ----
                for oh in range(2):
                    for jc in range(4):
                        nc.tensor.matmul(
                            psc[oh * 4 + jc][:],
                            lhsT=wdefT[:, k, 128 * oh:128 * (oh + 1)],
                            rhs=valT[:, 4 * jc:4 * (jc + 1), :],
                            start=(k == 0), stop=(k == NK - 1))

            if phase != 3:
                continue
            for oh in range(2):
                for jc in range(4):
                    yo = opool.tile([128, 512], F32, tag='yo')
                    nc.scalar.activation(
                        yo[:], psc[oh * 4 + jc][:], ACTF.Relu,
                        bias=bdef[:, oh:oh + 1], scale=1.0)
                    j0 = half * JH + 512 * jc
                    nc.sync.dma_start(
                        y_d.ap()[128 * oh:128 * (oh + 1), j0:j0 + 512],
                        yo[:])


def _host_prep(x, w_off, b_off, w_def, b_def):
    """Build per-core input maps."""
    x = np.asarray(x, np.float32)
    w_off = np.asarray(w_off, np.float32)
    b_off = np.asarray(b_off, np.float32)
    w_def = np.asarray(w_def, np.float32)
    b_def = np.asarray(b_def, np.float32)

    woffT = np.stack([w_off[:, :, s // 3, s % 3].T for s in range(9)])
    woffT = _to_bf16(np.ascontiguousarray(woffT, np.float32))  # [9, 128, 18]
    wdefT = np.stack([w_def[:, :, s // 3, s % 3].T for s in range(9)])
    wdefT = _to_bf16(np.ascontiguousarray(wdefT))             # [9, 128, 256]
    bdef2 = np.ascontiguousarray(b_def.reshape(2, 128).T)     # [128, 2]
    ident = np.eye(128, dtype=np.float32)

    jp = np.arange(128)[:, None, None]
    jt = np.arange(NJT)[None, :, None]
    kk = np.arange(NK)[None, None, :]
    j = jt * 128 + jp
    ky = np.array(KY, np.float32)[kk]
    kx = np.array(KX, np.float32)[kk]
    hgk = (j // 64).astype(np.float32) + ky
    wgk = (j % 64).astype(np.float32) + kx
    hgk = np.ascontiguousarray(np.broadcast_to(hgk, (128, NJT, NK)), np.float32)
    wgk = np.ascontiguousarray(np.broadcast_to(wgk, (128, NJT, NK)), np.float32)

    xp = np.pad(x, ((0, 0), (0, 0), (1, 1), (1, 1))).reshape(B, CIN, 66 * 66)
    xpb = _to_bf16(xp)

    shared = {
        'woffT': woffT,
        'boff': np.ascontiguousarray(b_off.reshape(18, 1)),
        'wdefT': wdefT,
        'bdef': bdef2,
        'ident': ident,
        'hgk': hgk,
        'wgk': wgk,
    }
    in_maps = []
    for b in range(B):
        m = dict(shared)
        m['xp'] = np.ascontiguousarray(xpb[b])
        in_maps.append(m)
    return in_maps


def _to_bf16(a):
    import ml_dtypes
    return a.astype(ml_dtypes.bfloat16)


LAST_RESULTS = None


def _ensure_trace_support():
    """Register the NTFF profile hook that the slim agent image lacks, and
    stub out the artifact upload. Only used when KBENCH_TRACE is set."""
    import contextlib
    import ctypes
    import types

    import concourse.bass_utils as bu
    bu.upload_artifacts = lambda tmpdir: tmpdir

    if 'antenv.axon_hooks' in sys.modules:
        return
    so_path = '/opt/axon/libaxon_pjrt.so'
    if not os.path.exists(so_path):
        return
    lib = ctypes.CDLL(so_path)
    if not hasattr(lib, 'axon_start_nrt_profile'):
        return
    lib.axon_start_nrt_profile.argtypes = [
        ctypes.POINTER(ctypes.c_int64), ctypes.c_size_t]
    lib.axon_start_nrt_profile.restype = ctypes.c_int64
    lib.axon_stop_nrt_profile.argtypes = [ctypes.c_char_p]
    lib.axon_stop_nrt_profile.restype = ctypes.c_int64

    @contextlib.contextmanager
    def _hook(output_dir, device_ids):
        import jax
        jax.devices()
        if device_ids:
            ids = (ctypes.c_int64 * len(device_ids))(*device_ids)
            rc = lib.axon_start_nrt_profile(ids, len(device_ids))
        else:
            rc = lib.axon_start_nrt_profile(None, 0)
        if rc != 0:
            raise RuntimeError(f'axon_start_nrt_profile rc={rc}')
        try:
            yield
        finally:
            n = lib.axon_stop_nrt_profile(str(output_dir).encode())
            print(f'profile: {n} file(s) written to {output_dir}',
                  file=sys.stderr)

    mod = types.ModuleType('antenv.axon_hooks')
    mod.get_axon_ntff_profile_hook = lambda: _hook
    mod.set_axon_ntff_profile_hook = lambda h: None
    sys.modules['antenv.axon_hooks'] = mod


def kernel(x, w_off, b_off, w_def, b_def):
    global LAST_RESULTS
    if 'nc' not in _CACHE:
        _CACHE['nc'] = _build_program(
            phase=int(os.environ.get('KBENCH_PHASE', '3')))
    nc = _CACHE['nc']
    in_maps = _host_prep(x, w_off, b_off, w_def, b_def)
    trace = bool(os.environ.get('KBENCH_TRACE'))
    if trace:
        _ensure_trace_support()
    res = run_bass_kernel_spmd(
        nc, in_maps, core_ids=list(range(B)),
        trace=trace,
    )
    LAST_RESULTS = res
    out = np.stack([res.results[b]['y'].reshape(COUT, H, W) for b in range(B)])
    return out.astype(np.float32)


# revision 11
# speedup vs baseline: 1.0312x; 1.0312x over previous
"""Trainium2 Bass kernel for nn_DeformableBlock (deformable conv v1 block).

Contract: kernel(**inputs) takes FULL unsharded inputs, returns FULL output.
Sharding: data-parallel over batch (B=8 -> 8 NeuronCores, 1 batch each),
weights replicated.

Per-core algorithm (one batch, Cin=128, Cout=256, H=W=64), v5:
  1. offset conv (3x3, pad 1) as 9 shifted bf16 matmuls -> offset [18, 4096]
  2. PE-transpose offsets to pixel-major; DVE computes, per (pixel, k):
     a 2x2 anchor (ay, bx) = clamp(floor-sample corner) and 4 tap weights
     w4 with clamp-swap logic (invalid taps get weight 0, data stays
     in-bounds), plus one gather index idx = ay*64 + bx in [0, 4030].
  3. idx is wrapped to the dma_gather layout (i%16 partitions) entirely
     on-chip with two rounds of PE transposes (no DRAM round trip, no
     xbar-fallback descriptor storm).
  4. Z table in DRAM: Z[j] = (xT[j], xT[j+64], xT[j+1], xT[j+65]) bf16 --
     1 KB per pixel holding the full 2x2 bilinear patch, so ONE gather
     descriptor per (k, pixel) fetches all 4 taps (halves Q7 SWDGE time).
  5. per (k, half): dma_gather -> G [128 jp, 16 jt, 512]; DVE tap weighting
     with per-partition scalars (ts + 3 stt) -> val [128 jp, 16, 128] bf16;
     one batched xbar DMA-transpose -> valT [128 c, 16, 128 jp] = channel-
     major j-natural.
  6. main conv: PSUM-resident accumulation over k: out[oh, jc] += W_k @ valT
     (8 PSUM banks = 2 oh x 4 jc chunks of [128, 512] f32 per half).
  7. bias + ReLU on ScalarE, DMA out [256, 4096] f32
"""
import os
import sys
import numpy as np

try:
    import concourse.bass as bass
except ImportError:  # pragma: no cover
    sys.path.insert(0, '/opt/trn_rl_repo')
    import concourse.bass as bass
import concourse.bacc as bacc

import concourse.mybir as mybir
import concourse.tile as tile
from concourse.bass_utils import run_bass_kernel_spmd

F32 = mybir.dt.float32
BF16 = mybir.dt.bfloat16
I32 = mybir.dt.int32
I16 = mybir.dt.int16
ALU = mybir.AluOpType
ACTF = mybir.ActivationFunctionType

B, CIN, COUT, H, W = 8, 128, 256, 64, 64
HW = H * W          # 4096
NJT = HW // 128     # 32 pixel-major tiles
NK = 9
KY = [(-1), (-1), (-1), 0, 0, 0, 1, 1, 1]
KX = [(-1), 0, 1, (-1), 0, 1, (-1), 0, 1]
NHALF = 2
JH = HW // NHALF    # 2048 pixels per half
ZW = 4 * CIN        # 512 elements per Z row (2x2 patch, channel-major taps)

_CACHE = {}


def _split_multiwaits(nc, max_waits=1, kinds=None):
    """walrus CoreV3 codegen rejects control instructions carrying more
    than one sem-wait; split the excess into a chain of same-engine
    drains placed directly before the offender."""
    if kinds is None:
        kinds = (mybir.InstDrain,)
    n_split = 0
    for fn in nc.m.functions:
        for bb in fn.blocks:
            insts = list(bb.instructions)
            new = []
            changed = False
            for inst in insts:
                si = inst.sync_info
                if (isinstance(inst, kinds) and si is not None
                        and len(si.on_wait) > max_waits):
                    waits = list(si.on_wait)
                    pre, rest = waits[:-max_waits], waits[-max_waits:]
                    for i in range(0, len(pre), max_waits):
                        chunk = pre[i:i + max_waits]
                        d = mybir.InstDrain(
                            name=f"{inst.name}-wsplit{i}",
                            engine=inst.engine,
                            ins=[], outs=[],
                            sync_info=mybir.SyncInfo(
                                on_wait=chunk, on_update=[]),
                        )
                        new.append(d)
                        n_split += 1
                    inst.sync_info = mybir.SyncInfo(
                        on_wait=rest, on_update=list(si.on_update))
                    changed = True
                new.append(inst)
            if changed:
                bb.instructions = new
    return n_split


def _build_program(phase=3):
    nc = bacc.Bacc('TRN2', target_bir_lowering=False, debug=False,
                   enable_asserts=False, num_devices=B)

    # ---- DRAM I/O ----
    xp_d = nc.dram_tensor('xp', [CIN, 66 * 66], BF16, kind='ExternalInput')
    woffT_d = nc.dram_tensor('woffT', [9, CIN, 18], BF16, kind='ExternalInput')
    boff_d = nc.dram_tensor('boff', [18, 1], F32, kind='ExternalInput')
    wdefT_d = nc.dram_tensor('wdefT', [NK, CIN, COUT], BF16, kind='ExternalInput')
    bdef_d = nc.dram_tensor('bdef', [128, 2], F32, kind='ExternalInput')
    ident_d = nc.dram_tensor('ident', [128, 128], F32, kind='ExternalInput')
    hgk_d = nc.dram_tensor('hgk', [128, NJT, NK], F32, kind='ExternalInput')
    wgk_d = nc.dram_tensor('wgk', [128, NJT, NK], F32, kind='ExternalInput')
    y_d = nc.dram_tensor('y', [COUT, HW], F32, kind='ExternalOutput')

    # DRAM scratch: Z[j] = (xT[j], xT[j+64], xT[j+1], xT[j+65])
    z_d = nc.dram_tensor('z_scratch', [HW, ZW], BF16, kind='Internal')

    with tile.TileContext(nc) as tc:
        with tc.tile_pool(name='const', bufs=1) as cpool:
            # persistent small tensors
            wdefT = cpool.tile([CIN, NK, COUT], BF16, tag='wdefT')
            nc.sync.dma_start(
                wdefT[:], wdefT_d.ap().rearrange('k c o -> c k o'))
            bdef = cpool.tile([128, 2], F32, tag='bdef')
            nc.sync.dma_start(bdef[:], bdef_d.ap())
            # wrapped gather indices: [p, k, half, fa, fb]; per (k, half) the
            # free slice (fa, fb) is the contiguous 128-entry idx/16 dim.
            idxwr = cpool.tile([128, NK, NHALF, 16, 8], I16, tag='idxwr')
            w4 = cpool.tile([128, NJT, NK, 4], F32, tag='w4')

            # ======== phase 1: offsets, indices, weights, Z ========
            with (
                tc.tile_pool(name='p1', bufs=1) as apool,
                tc.tile_pool(name='ps_small', bufs=2, space='PSUM') as ps_small,
            ):
                xbf = apool.tile([CIN, 66 * 66], BF16, tag='xbf')
                nc.sync.dma_start(xbf[:], xp_d.ap())
                ident = apool.tile([128, 128], F32, tag='ident')
                nc.sync.dma_start(ident[:], ident_d.ap())
                identb = apool.tile([128, 128], BF16, tag='identb')
                nc.vector.tensor_copy(identb[:], ident[:])
                woffT = apool.tile([CIN, 9, 18], BF16, tag='woffT')
                nc.sync.dma_start(
                    woffT[:], woffT_d.ap().rearrange('s c o -> c s o'))
                boff = apool.tile([18, 1], F32, tag='boff')
                nc.sync.dma_start(boff[:], boff_d.ap())
                hgk = apool.tile([128, NJT, NK], F32, tag='hgk')
                nc.sync.dma_start(hgk[:], hgk_d.ap())
                wgk = apool.tile([128, NJT, NK], F32, tag='wgk')
                nc.sync.dma_start(wgk[:], wgk_d.ap())

                # ---------- xT build first: its PE transposes are cheap and
                # unblock the Z-build DMAs, which then run under the offset
                # conv + index math (first gather needs Z complete). --------
                xc = apool.tile([CIN, HW], BF16, tag='xc')
                xin_all = bass.AP(
                    tensor=xbf[:].tensor,
                    offset=xbf[:].offset + 67,
                    ap=[list(xbf[:].ap[0]), [66, 64], [1, 64]])
                nc.vector.tensor_copy(xc[:], xin_all)
                xTsb = apool.tile([128, NJT, CIN], BF16, tag='xTsb')
                for jt in range(NJT):
                    ps = ps_small.tile([128, 128], BF16, tag='psb', name='ps')
                    nc.tensor.transpose(
                        ps[:], xc[:, 128 * jt:128 * (jt + 1)], identb[:])
                    nc.vector.tensor_copy(xTsb[:, jt, :], ps[:])

                # ---------- Z build (7 DMAs; row j holds the 2x2 patch) ----
                # Z[:, blk] = xT[j + d], d in (0, 64, 1, 65).  Source rows
                # come from xTsb with a partition shift; rows of Z that
                # would read past xT (j + d >= 4096) are never gathered
                # (idx <= 4030), so they are simply skipped.
                def zdst(blk, p0, p1, jt0, jt1):
                    full = z_d.ap()[:, 128 * blk:128 * (blk + 1)].rearrange(
                        '(jt jp) c -> jp jt c', jp=128)
                    return full[p0:p1, jt0:jt1, :]

                # blk 0: d=0
                nc.sync.dma_start(zdst(0, 0, 128, 0, 32), xTsb[:])
                # blk 1: d=64 -> src (jp+64, jt) / (jp-64, jt+1)
                nc.sync.dma_start(zdst(1, 0, 64, 0, 32), xTsb[64:128, :, :])
                nc.sync.dma_start(zdst(1, 64, 128, 0, 31), xTsb[0:64, 1:32, :])
                # blk 2: d=1 -> src (jp+1, jt) / (0, jt+1)
                nc.sync.dma_start(zdst(2, 0, 127, 0, 32), xTsb[1:128, :, :])
                nc.sync.dma_start(zdst(2, 127, 128, 0, 31), xTsb[0:1, 1:32, :])
                # blk 3: d=65 -> src (jp+65, jt) / (jp-63, jt+1)
                nc.sync.dma_start(zdst(3, 0, 63, 0, 32), xTsb[65:128, :, :])
                nc.sync.dma_start(zdst(3, 63, 128, 0, 31), xTsb[0:65, 1:32, :])

                # ---------- offset conv: off [18, 4096] f32 ----------
                # Conv runs on the padded 66-wide grid so the streaming
                # operand is a single contiguous run; the interior is
                # extracted in the ACT epilogue.
                off_sb = apool.tile([18, HW], F32, tag='off_sb')
                chunks = [(1 + 7 * i, 7) for i in range(9)] + [(64, 1)]
                for r0, nr in chunks:
                    nfree = 66 * (nr - 1) + 64
                    ps = ps_small.tile([18, 512], F32, tag='ps', name='ps')
                    for s in range(9):
                        dh, dw = s // 3, s % 3
                        beg = r0 * 66 + 1 + (dh - 1) * 66 + (dw - 1)
                        rhs = bass.AP(
                            tensor=xbf[:].tensor,
                            offset=xbf[:].offset + beg,
                            ap=[list(xbf[:].ap[0]), [1, nfree]],
                        )
                        nc.tensor.matmul(
                            ps[:, :nfree], lhsT=woffT[:, s, :], rhs=rhs,
                            start=(s == 0), stop=(s == 8))
                    src_in = bass.AP(
                        tensor=ps[:].tensor, offset=ps[:].offset,
                        ap=[list(ps[:].ap[0]), [66, nr], [1, 64]])
                    nc.scalar.activation(
                        off_sb[:, 64 * (r0 - 1):64 * (r0 - 1 + nr)], src_in,
                        ACTF.Identity, bias=boff[:], scale=1.0)

                # ---------- transpose offsets to pixel-major ----------
                offT = apool.tile([128, NJT, 18], F32, tag='offT')
                for jt in range(NJT):
                    ps = ps_small.tile([128, 18], F32, tag='ps')
                    nc.tensor.transpose(
                        ps[:], off_sb[:, 128 * jt:128 * (jt + 1)],
                        ident[:18, :18])
                    nc.vector.tensor_copy(offT[:, jt, :], ps[:])

                # ---------- index/weight arithmetic (pixel-major) ----------
                sh = [128, NJT, NK]

                def T(tag, dt=F32):
                    return apool.tile(sh, dt, tag=tag, name=tag)

                dyx = offT[:].rearrange('p jt (k two) -> p jt k two', two=2)
                dy = dyx[:, :, :, 0]
                dx = dyx[:, :, :, 1]

                ti = apool.tile(sh, I32, tag='ti')
                fdy, fdx = T('fdy'), T('fdx')
                tmp1, tmp2, tmp3 = T('tmp1'), T('tmp2'), T('tmp3')
                # floor(dy)
                nc.vector.tensor_copy(ti[:], dy)
                nc.vector.tensor_copy(fdy[:], ti[:])
                nc.vector.tensor_tensor(tmp1[:], fdy[:], dy, ALU.is_gt)
                nc.vector.tensor_tensor(fdy[:], fdy[:], tmp1[:], ALU.subtract)
                # floor(dx)
                nc.vector.tensor_copy(ti[:], dx)
                nc.vector.tensor_copy(fdx[:], ti[:])
                nc.vector.tensor_tensor(tmp1[:], fdx[:], dx, ALU.is_gt)
                nc.vector.tensor_tensor(fdx[:], fdx[:], tmp1[:], ALU.subtract)

                ly, lx = T('ly'), T('lx')
                nc.vector.tensor_tensor(ly[:], dy, fdy[:], ALU.subtract)
                nc.vector.tensor_tensor(lx[:], dx, fdx[:], ALU.subtract)

                y0, x0 = T('y0'), T('x0')
                nc.vector.tensor_tensor(y0[:], hgk[:], fdy[:], ALU.add)
                nc.vector.tensor_tensor(x0[:], wgk[:], fdx[:], ALU.add)

                # y anchor + tap-swap terms
                ay, y01, ayp = T('ay'), T('y01'), T('ayp')
                f0, f1, f3 = T('f0'), T('f1'), T('f3')
                nc.vector.tensor_scalar(ay[:], y0[:], 62.0, 0.0, ALU.min, ALU.max)
                nc.vector.tensor_scalar(y01[:], y0[:], 1.0, None, ALU.add)
                nc.vector.tensor_scalar(ayp[:], ay[:], 1.0, None, ALU.add)
                nc.vector.tensor_tensor(f0[:], y0[:], ay[:], ALU.is_equal)
                nc.vector.tensor_tensor(f1[:], y01[:], ay[:], ALU.is_equal)
                nc.vector.tensor_tensor(f3[:], y0[:], ayp[:], ALU.is_equal)

                # x anchor + tap-swap terms
                bx, x01, bxp = T('bx'), T('x01'), T('bxp')
                e0, e1, e3 = T('e0'), T('e1'), T('e3')
                nc.vector.tensor_scalar(bx[:], x0[:], 62.0, 0.0, ALU.min, ALU.max)
                nc.vector.tensor_scalar(x01[:], x0[:], 1.0, None, ALU.add)
                nc.vector.tensor_scalar(bxp[:], bx[:], 1.0, None, ALU.add)
                nc.vector.tensor_tensor(e0[:], x0[:], bx[:], ALU.is_equal)
                nc.vector.tensor_tensor(e1[:], x01[:], bx[:], ALU.is_equal)
                nc.vector.tensor_tensor(e3[:], x0[:], bxp[:], ALU.is_equal)

                # wyA = (1-ly)f0 + ly*f1 ; wyB = ly*f0 + (1-ly)*f3
                wyA, wyB = T('wyA'), T('wyB')
                nc.vector.tensor_tensor(tmp1[:], ly[:], f0[:], ALU.mult)
                nc.vector.tensor_tensor(wyA[:], f0[:], tmp1[:], ALU.subtract)
                nc.vector.tensor_tensor(tmp2[:], ly[:], f1[:], ALU.mult)
                nc.vector.tensor_tensor(wyA[:], wyA[:], tmp2[:], ALU.add)
                nc.vector.tensor_tensor(tmp2[:], ly[:], f3[:], ALU.mult)
                nc.vector.tensor_tensor(tmp3[:], f3[:], tmp2[:], ALU.subtract)
                nc.vector.tensor_tensor(wyB[:], tmp1[:], tmp3[:], ALU.add)

                wxA, wxB = T('wxA'), T('wxB')
                nc.vector.tensor_tensor(tmp1[:], lx[:], e0[:], ALU.mult)
                nc.vector.tensor_tensor(wxA[:], e0[:], tmp1[:], ALU.subtract)
                nc.vector.tensor_tensor(tmp2[:], lx[:], e1[:], ALU.mult)
                nc.vector.tensor_tensor(wxA[:], wxA[:], tmp2[:], ALU.add)
                nc.vector.tensor_tensor(tmp2[:], lx[:], e3[:], ALU.mult)
                nc.vector.tensor_tensor(tmp3[:], e3[:], tmp2[:], ALU.subtract)
                nc.vector.tensor_tensor(wxB[:], tmp1[:], tmp3[:], ALU.add)

                # tap weights (Z block order: (yA,xA),(yB,xA),(yA,xB),(yB,xB))
                nc.vector.tensor_tensor(w4[:, :, :, 0], wyA[:], wxA[:], ALU.mult)
                nc.vector.tensor_tensor(w4[:, :, :, 1], wyB[:], wxA[:], ALU.mult)
                nc.vector.tensor_tensor(w4[:, :, :, 2], wyA[:], wxB[:], ALU.mult)
                nc.vector.tensor_tensor(w4[:, :, :, 3], wyB[:], wxB[:], ALU.mult)

                # gather index idx = ay*64 + bx  (exact int in f32, <= 4030)
                idxf = T('idxf')
                nc.vector.tensor_scalar(tmp1[:], ay[:], 64.0, None, ALU.mult)
                nc.vector.tensor_tensor(idxf[:], tmp1[:], bx[:], ALU.add)

                # ---------- on-chip idx wrap (PE transposes) ----------
                # Round 1: pixel-major [jp, jt] -> j-natural rows [jt, jp].
                idxT = apool.tile([NJT, NK, 128], F32, tag='idxT')
                for k in range(NK):
                    ps = ps_small.tile([NJT, 128], F32, tag='ps')
                    nc.tensor.transpose(ps[:], idxf[:, :, k], ident[:])
                    nc.vector.tensor_copy(idxT[:, k, :], ps[:])
                # Round 2: [jt, fb*16+p] -> [p, (q, fa=jtl)] per (k, fb),
                # cast to i16 on the PSUM evacuation.
                for k in range(NK):
                    for fb in range(8):
                        ps = ps_small.tile([16, NJT], F32, tag='ps16',
                                           name='ps16')
                        nc.tensor.transpose(
                            ps[:], idxT[:, k, 16 * fb:16 * (fb + 1)],
                            ident[:NJT, :NJT])
                        nc.vector.tensor_copy(
                            idxwr[0:16, k, :, :, fb],
                            ps[:].rearrange('p (h a) -> p h a', h=NHALF))
                # replicate 16 -> 128 partitions (3 doubling copies)
                for g in (16, 32, 64):
                    nc.sync.dma_start(idxwr[g:2 * g], idxwr[0:g])

            # ======== phase 2: gather + weighting + transpose + conv ========
            if phase != 1:
                _phase2(nc, tc, wdefT, bdef, idxwr, w4, z_d, y_d, phase)
            if phase != 3:
                with tc.tile_pool(name='zz', bufs=1) as zp:
                    zt = zp.tile([128, HW], F32, tag='zt')
                    nc.vector.memset(zt[:], 0)
                    for oh in range(2):
                        nc.sync.dma_start(
                            y_d.ap()[128 * oh:128 * (oh + 1), :], zt[:])

    nc.finalize()
    _split_multiwaits(nc)
    return nc


def _phase2(nc, tc, wdefT, bdef, idxwr, w4, z_d, y_d, phase=3):
    # Tap weighting: ScalarE produces the last tap's product (per-partition
    # scale, own SBUF ports); DVE runs three scalar_tensor_tensor ops, the
    # first absorbing the ACT product as its in1.  stt overlaps SWDGE gather
    # descriptor generation nearly cleanly (measured), unlike tensor_scalar
    # whose 2-port perf mode serializes against the Q7 shared-port lock.
    with (
        tc.tile_pool(name='gath', bufs=2) as gpool,
        tc.tile_pool(name='tmpa', bufs=4) as tpool,
        tc.tile_pool(name='val', bufs=2) as vpool,
        tc.tile_pool(name='valT', bufs=3) as vtpool,
        tc.tile_pool(name='outp', bufs=2) as opool,
        tc.tile_pool(name='pcv', bufs=1, space='PSUM') as pcv,
    ):
        for half in range(NHALF):
            if phase == 3:
                psc = [pcv.tile([128, 512], F32, tag=f'psc{i}',
                                name=f'psc{i}') for i in range(8)]
            for k in range(NK):
                # ---- gather: one 1KB descriptor per pixel, all 4 taps ----
                G = gpool.tile([128, JH // 128, ZW], BF16, tag='G', name='G')
                in_ap = bass.AP(
                    tensor=z_d, offset=0,
                    ap=[[ZW, HW], [1, ZW]],
                )
                nc.gpsimd.dma_gather(
                    out_ap=G[:],
                    in_ap=in_ap,
                    idxs_ap=idxwr[:, k, half, :, :],
                    num_idxs=JH,
                    num_idxs_reg=JH,
                    elem_size=ZW,
                    elem_step=ZW,
                    transpose=False,
                    single_packet=False,
                )
                if phase == 21:
                    continue

                # ---- tap weighting (pixel-major, per-partition scalars) ----
                val = vpool.tile([128, JH // 128, CIN], BF16, tag='val',
                                 name='val')
                for jtl in range(JH // 128):
                    jt = half * (JH // 128) + jtl
                    vsl = val[:, jtl, :]
                    tmpa = tpool.tile([128, CIN], BF16, tag='tmpa',
                                      name='tmpa')
                    nc.scalar.mul(tmpa[:], G[:, jtl, 3 * CIN:4 * CIN],
                                  w4[:, jt, k, 3].unsqueeze(1))
                    nc.vector.scalar_tensor_tensor(
                        vsl, G[:, jtl, 0:CIN],
                        w4[:, jt, k, 0].unsqueeze(1), tmpa[:],
                        ALU.mult, ALU.add)
                    for t in (1, 2):
                        nc.vector.scalar_tensor_tensor(
                            vsl, G[:, jtl, t * CIN:(t + 1) * CIN],
                            w4[:, jt, k, t].unsqueeze(1), vsl,
                            ALU.mult, ALU.add)

                # ---- batched xbar transpose -> channel-major j-natural ----
                valT = vtpool.tile([128, JH // 128, 128], BF16, tag='valT',
                                   name='valT')
                nc.sync.dma_start_transpose(valT[:], val[:])

                if phase != 3:
                    continue
                # ---- main conv: accumulate over k in PSUM ----
                for oh in range(2):
                    for jc in range(4):
                        nc.tensor.matmul(
                            psc[oh * 4 + jc][:],
                            lhsT=wdefT[:, k, 128 * oh:128 * (oh + 1)],
                            rhs=valT[:, 4 * jc:4 * (jc + 1), :],
                            start=(k == 0), stop=(k == NK - 1))

            if phase != 3:
                continue
            for oh in range(2):
                for jc in range(4):
                    yo = opool.tile([128, 512], F32, tag='yo')
                    nc.scalar.activation(
                        yo[:], psc[oh * 4 + jc][:], ACTF.Relu,
                        bias=bdef[:, oh:oh + 1], scale=1.0)
                    j0 = half * JH + 512 * jc
                    nc.sync.dma_start(
                        y_d.ap()[128 * oh:128 * (oh + 1), j0:j0 + 512],
                        yo[:])


def _host_prep(x, w_off, b_off, w_def, b_def):
    """Build per-core input maps."""
    x = np.asarray(x, np.float32)
    w_off = np.asarray(w_off, np.float32)
    b_off = np.asarray(b_off, np.float32)
    w_def = np.asarray(w_def, np.float32)
    b_def = np.asarray(b_def, np.float32)

    woffT = np.stack([w_off[:, :, s // 3, s % 3].T for s in range(9)])
    woffT = _to_bf16(np.ascontiguousarray(woffT, np.float32))  # [9, 128, 18]
    wdefT = np.stack([w_def[:, :, s // 3, s % 3].T for s in range(9)])
    wdefT = _to_bf16(np.ascontiguousarray(wdefT))             # [9, 128, 256]
    bdef2 = np.ascontiguousarray(b_def.reshape(2, 128).T)     # [128, 2]
    ident = np.eye(128, dtype=np.float32)

    jp = np.arange(128)[:, None, None]
    jt = np.arange(NJT)[None, :, None]
    kk = np.arange(NK)[None, None, :]
    j = jt * 128 + jp
    ky = np.array(KY, np.float32)[kk]
    kx = np.array(KX, np.float32)[kk]
    hgk = (j // 64).astype(np.float32) + ky
    wgk = (j % 64).astype(np.float32) + kx
    hgk = np.ascontiguousarray(np.broadcast_to(hgk, (128, NJT, NK)), np.float32)
    wgk = np.ascontiguousarray(np.broadcast_to(wgk, (128, NJT, NK)), np.float32)

    xp = np.pad(x, ((0, 0), (0, 0), (1, 1), (1, 1))).reshape(B, CIN, 66 * 66)
    xpb = _to_bf16(xp)

    shared = {
        'woffT': woffT,
        'boff': np.ascontiguousarray(b_off.reshape(18, 1)),
        'wdefT': wdefT,
        'bdef': bdef2,
        'ident': ident,
        'hgk': hgk,
        'wgk': wgk,
    }
    in_maps = []
    for b in range(B):
        m = dict(shared)
        m['xp'] = np.ascontiguousarray(xpb[b])
        in_maps.append(m)
    return in_maps


def _to_bf16(a):
    import ml_dtypes
    return a.astype(ml_dtypes.bfloat16)


LAST_RESULTS = None


def _ensure_trace_support():
    """Register the NTFF profile hook that the slim agent image lacks, and
    stub out the artifact upload. Only used when KBENCH_TRACE is set."""
    import contextlib
    import ctypes
    import types

    import concourse.bass_utils as bu
    bu.upload_artifacts = lambda tmpdir: tmpdir

    if 'antenv.axon_hooks' in sys.modules:
        return
    so_path = '/opt/axon/libaxon_pjrt.so'
    if not os.path.exists(so_path):
        return
    lib = ctypes.CDLL(so_path)
    if not hasattr(lib, 'axon_start_nrt_profile'):
        return
    lib.axon_start_nrt_profile.argtypes = [
        ctypes.POINTER(ctypes.c_int64), ctypes.c_size_t]
    lib.axon_start_nrt_profile.restype = ctypes.c_int64
    lib.axon_stop_nrt_profile.argtypes = [ctypes.c_char_p]
    lib.axon_stop_nrt_profile.restype = ctypes.c_int64

    @contextlib.contextmanager
    def _hook(output_dir, device_ids):
        import jax
        jax.devices()
        if device_ids:
            ids = (ctypes.c_int64 * len(device_ids))(*device_ids)
            rc = lib.axon_start_nrt_profile(ids, len(device_ids))
        else:
            rc = lib.axon_start_nrt_profile(None, 0)
        if rc != 0:
            raise RuntimeError(f'axon_start_nrt_profile rc={rc}')
        try:
            yield
        finally:
            n = lib.axon_stop_nrt_profile(str(output_dir).encode())
            print(f'profile: {n} file(s) written to {output_dir}',
                  file=sys.stderr)

    mod = types.ModuleType('antenv.axon_hooks')
    mod.get_axon_ntff_profile_hook = lambda: _hook
    mod.set_axon_ntff_profile_hook = lambda h: None
    sys.modules['antenv.axon_hooks'] = mod


def kernel(x, w_off, b_off, w_def, b_def):
    global LAST_RESULTS
    if 'nc' not in _CACHE:
        _CACHE['nc'] = _build_program(
            phase=int(os.environ.get('KBENCH_PHASE', '3')))
    nc = _CACHE['nc']
    in_maps = _host_prep(x, w_off, b_off, w_def, b_def)
    trace = bool(os.environ.get('KBENCH_TRACE'))
    if trace:
        _ensure_trace_support()
    res = run_bass_kernel_spmd(
        nc, in_maps, core_ids=list(range(B)),
        trace=trace,
    )
    LAST_RESULTS = res
    out = np.stack([res.results[b]['y'].reshape(COUT, H, W) for b in range(B)])
    return out.astype(np.float32)


# revision 14
# speedup vs baseline: 1.0775x; 1.0450x over previous
"""Trainium2 Bass kernel for nn_DeformableBlock (deformable conv v1 block).

Contract: kernel(**inputs) takes FULL unsharded inputs, returns FULL output.
Sharding: data-parallel over batch (B=8 -> 8 NeuronCores, 1 batch each),
weights replicated.

Per-core algorithm (one batch, Cin=128, Cout=256, H=W=64), v5:
  1. offset conv (3x3, pad 1) as 9 shifted bf16 matmuls -> offset [18, 4096]
  2. PE-transpose offsets to pixel-major; DVE computes, per (pixel, k):
     a 2x2 anchor (ay, bx) = clamp(floor-sample corner) and 4 tap weights
     w4 with clamp-swap logic (invalid taps get weight 0, data stays
     in-bounds), plus one gather index idx = ay*64 + bx in [0, 4030].
  3. idx is wrapped to the dma_gather layout (i%16 partitions) entirely
     on-chip with two rounds of PE transposes (no DRAM round trip, no
     xbar-fallback descriptor storm).
  4. Z table in DRAM: Z[j] = (xT[j], xT[j+64], xT[j+1], xT[j+65]) bf16 --
     1 KB per pixel holding the full 2x2 bilinear patch, so ONE gather
     descriptor per (k, pixel) fetches all 4 taps (halves Q7 SWDGE time).
  5. per (k, half): dma_gather -> G [128 jp, 16 jt, 512]; DVE tap weighting
     with per-partition scalars (ts + 3 stt) -> val [128 jp, 16, 128] bf16;
     one batched xbar DMA-transpose -> valT [128 c, 16, 128 jp] = channel-
     major j-natural.
  6. main conv: PSUM-resident accumulation over k: out[oh, jc] += W_k @ valT
     (8 PSUM banks = 2 oh x 4 jc chunks of [128, 512] f32 per half).
  7. bias + ReLU on ScalarE, DMA out [256, 4096] f32
"""
import os
import sys
import numpy as np

try:
    import concourse.bass as bass
except ImportError:  # pragma: no cover
    sys.path.insert(0, '/opt/trn_rl_repo')
    import concourse.bass as bass
import concourse.bacc as bacc

import concourse.mybir as mybir
import concourse.tile as tile
from concourse.bass_utils import run_bass_kernel_spmd

F32 = mybir.dt.float32
BF16 = mybir.dt.bfloat16
I32 = mybir.dt.int32
I16 = mybir.dt.int16
ALU = mybir.AluOpType
ACTF = mybir.ActivationFunctionType

B, CIN, COUT, H, W = 8, 128, 256, 64, 64
HW = H * W          # 4096
NJT = HW // 128     # 32 pixel-major tiles
NK = 9
KY = [(-1), (-1), (-1), 0, 0, 0, 1, 1, 1]
KX = [(-1), 0, 1, (-1), 0, 1, (-1), 0, 1]
NHALF = 2
JH = HW // NHALF    # 2048 pixels per half
ZW = 4 * CIN        # 512 elements per Z row (2x2 patch, channel-major taps)

_CACHE = {}


def _split_multiwaits(nc, max_waits=1, kinds=None):
    """walrus CoreV3 codegen rejects control instructions carrying more
    than one sem-wait; split the excess into a chain of same-engine
    drains placed directly before the offender."""
    if kinds is None:
        kinds = (mybir.InstDrain,)
    n_split = 0
    for fn in nc.m.functions:
        for bb in fn.blocks:
            insts = list(bb.instructions)
            new = []
            changed = False
            for inst in insts:
                si = inst.sync_info
                if (isinstance(inst, kinds) and si is not None
                        and len(si.on_wait) > max_waits):
                    waits = list(si.on_wait)
                    pre, rest = waits[:-max_waits], waits[-max_waits:]
                    for i in range(0, len(pre), max_waits):
                        chunk = pre[i:i + max_waits]
                        d = mybir.InstDrain(
                            name=f"{inst.name}-wsplit{i}",
                            engine=inst.engine,
                            ins=[], outs=[],
                            sync_info=mybir.SyncInfo(
                                on_wait=chunk, on_update=[]),
                        )
                        new.append(d)
                        n_split += 1
                    inst.sync_info = mybir.SyncInfo(
                        on_wait=rest, on_update=list(si.on_update))
                    changed = True
                new.append(inst)
            if changed:
                bb.instructions = new
    return n_split


def _build_program(phase=3):
    nc = bacc.Bacc('TRN2', target_bir_lowering=False, debug=False,
                   enable_asserts=False, num_devices=B)

    # ---- DRAM I/O ----
    xp_d = nc.dram_tensor('xp', [CIN, 66 * 66], BF16, kind='ExternalInput')
    woffT_d = nc.dram_tensor('woffT', [9, CIN, 18], BF16, kind='ExternalInput')
    boff_d = nc.dram_tensor('boff', [18, 1], F32, kind='ExternalInput')
    wdefT_d = nc.dram_tensor('wdefT', [NK, CIN, COUT], BF16, kind='ExternalInput')
    bdef_d = nc.dram_tensor('bdef', [128, 2], F32, kind='ExternalInput')
    ident_d = nc.dram_tensor('ident', [128, 128], F32, kind='ExternalInput')
    hgk_d = nc.dram_tensor('hgk', [128, NJT, NK], F32, kind='ExternalInput')
    wgk_d = nc.dram_tensor('wgk', [128, NJT, NK], F32, kind='ExternalInput')
    y_d = nc.dram_tensor('y', [COUT, HW], F32, kind='ExternalOutput')

    # DRAM scratch: Z[j] = (xT[j], xT[j+64], xT[j+1], xT[j+65])
    z_d = nc.dram_tensor('z_scratch', [HW, ZW], BF16, kind='Internal')

    with tile.TileContext(nc) as tc:
        with tc.tile_pool(name='const', bufs=1) as cpool:
            # persistent small tensors
            wdefT = cpool.tile([CIN, NK, COUT], BF16, tag='wdefT')
            nc.sync.dma_start(
                wdefT[:], wdefT_d.ap().rearrange('k c o -> c k o'))
            bdef = cpool.tile([128, 2], F32, tag='bdef')
            nc.sync.dma_start(bdef[:], bdef_d.ap())
            # wrapped gather indices: [p, k, half, fa, fb]; per (k, half) the
            # free slice (fa, fb) is the contiguous 128-entry idx/16 dim.
            idxwr = cpool.tile([128, NK, NHALF, 16, 8], I16, tag='idxwr')
            w4 = cpool.tile([128, NJT, NK, 4], F32, tag='w4')

            # ======== phase 1: offsets, indices, weights, Z ========
            with (
                tc.tile_pool(name='p1', bufs=1) as apool,
                tc.tile_pool(name='ps_small', bufs=2, space='PSUM') as ps_small,
            ):
                xbf = apool.tile([CIN, 66 * 66], BF16, tag='xbf')
                nc.sync.dma_start(xbf[:], xp_d.ap())
                ident = apool.tile([128, 128], F32, tag='ident')
                nc.sync.dma_start(ident[:], ident_d.ap())
                identb = apool.tile([128, 128], BF16, tag='identb')
                nc.vector.tensor_copy(identb[:], ident[:])
                woffT = apool.tile([CIN, 9, 18], BF16, tag='woffT')
                nc.sync.dma_start(
                    woffT[:], woffT_d.ap().rearrange('s c o -> c s o'))
                boff = apool.tile([18, 1], F32, tag='boff')
                nc.sync.dma_start(boff[:], boff_d.ap())
                hgk = apool.tile([128, NJT, NK], F32, tag='hgk')
                nc.sync.dma_start(hgk[:], hgk_d.ap())
                wgk = apool.tile([128, NJT, NK], F32, tag='wgk')
                nc.sync.dma_start(wgk[:], wgk_d.ap())

                # ---------- xT build first: its PE transposes are cheap and
                # unblock the Z-build DMAs, which then run under the offset
                # conv + index math (first gather needs Z complete). --------
                xc = apool.tile([CIN, HW], BF16, tag='xc')
                xin_all = bass.AP(
                    tensor=xbf[:].tensor,
                    offset=xbf[:].offset + 67,
                    ap=[list(xbf[:].ap[0]), [66, 64], [1, 64]])
                nc.vector.tensor_copy(xc[:], xin_all)
                xTsb = apool.tile([128, NJT, CIN], BF16, tag='xTsb')
                for jt in range(NJT):
                    ps = ps_small.tile([128, 128], BF16, tag='psb', name='ps')
                    nc.tensor.transpose(
                        ps[:], xc[:, 128 * jt:128 * (jt + 1)], identb[:])
                    nc.scalar.copy(xTsb[:, jt, :], ps[:])

                # ---------- Z build (7 DMAs; row j holds the 2x2 patch) ----
                # Z[:, blk] = xT[j + d], d in (0, 64, 1, 65).  Source rows
                # come from xTsb with a partition shift; rows of Z that
                # would read past xT (j + d >= 4096) are never gathered
                # (idx <= 4030), so they are simply skipped.
                def zdst(blk, p0, p1, jt0, jt1):
                    full = z_d.ap()[:, 128 * blk:128 * (blk + 1)].rearrange(
                        '(jt jp) c -> jp jt c', jp=128)
                    return full[p0:p1, jt0:jt1, :]

                # blk 0: d=0
                nc.sync.dma_start(zdst(0, 0, 128, 0, 32), xTsb[:])
                # blk 1: d=64 -> src (jp+64, jt) / (jp-64, jt+1)
                nc.sync.dma_start(zdst(1, 0, 64, 0, 32), xTsb[64:128, :, :])
                nc.sync.dma_start(zdst(1, 64, 128, 0, 31), xTsb[0:64, 1:32, :])
                # blk 2: d=1 -> src (jp+1, jt) / (0, jt+1)
                nc.sync.dma_start(zdst(2, 0, 127, 0, 32), xTsb[1:128, :, :])
                nc.sync.dma_start(zdst(2, 127, 128, 0, 31), xTsb[0:1, 1:32, :])
                # blk 3: d=65 -> src (jp+65, jt) / (jp-63, jt+1)
                nc.sync.dma_start(zdst(3, 0, 63, 0, 32), xTsb[65:128, :, :])
                nc.sync.dma_start(zdst(3, 63, 128, 0, 31), xTsb[0:65, 1:32, :])

                # ---------- offset conv: off [18, 4096] f32 ----------
                # Conv runs on the padded 66-wide grid so the streaming
                # operand is a single contiguous run; the interior is
                # extracted in the ACT epilogue.
                off_sb = apool.tile([18, HW], F32, tag='off_sb')
                chunks = [(1 + 7 * i, 7) for i in range(9)] + [(64, 1)]
                for r0, nr in chunks:
                    nfree = 66 * (nr - 1) + 64
                    ps = ps_small.tile([18, 512], F32, tag='ps', name='ps')
                    for s in range(9):
                        dh, dw = s // 3, s % 3
                        beg = r0 * 66 + 1 + (dh - 1) * 66 + (dw - 1)
                        rhs = bass.AP(
                            tensor=xbf[:].tensor,
                            offset=xbf[:].offset + beg,
                            ap=[list(xbf[:].ap[0]), [1, nfree]],
                        )
                        nc.tensor.matmul(
                            ps[:, :nfree], lhsT=woffT[:, s, :], rhs=rhs,
                            start=(s == 0), stop=(s == 8))
                    src_in = bass.AP(
                        tensor=ps[:].tensor, offset=ps[:].offset,
                        ap=[list(ps[:].ap[0]), [66, nr], [1, 64]])
                    nc.scalar.activation(
                        off_sb[:, 64 * (r0 - 1):64 * (r0 - 1 + nr)], src_in,
                        ACTF.Identity, bias=boff[:], scale=1.0)

                # ---------- transpose offsets to pixel-major ----------
                offT = apool.tile([128, NJT, 18], F32, tag='offT')
                for jt in range(NJT):
                    ps = ps_small.tile([128, 18], F32, tag='ps')
                    nc.tensor.transpose(
                        ps[:], off_sb[:, 128 * jt:128 * (jt + 1)],
                        ident[:18, :18])
                    nc.scalar.copy(offT[:, jt, :], ps[:])

                # ---------- index/weight arithmetic (pixel-major) ----------
                sh = [128, NJT, NK]

                def T(tag, dt=F32):
                    return apool.tile(sh, dt, tag=tag, name=tag)

                dyx = offT[:].rearrange('p jt (k two) -> p jt k two', two=2)
                dy = dyx[:, :, :, 0]
                dx = dyx[:, :, :, 1]

                ti = apool.tile(sh, I32, tag='ti')
                fdy, fdx = T('fdy'), T('fdx')
                tmp1, tmp2, tmp3 = T('tmp1'), T('tmp2'), T('tmp3')
                # floor(dy)
                nc.vector.tensor_copy(ti[:], dy)
                nc.vector.tensor_copy(fdy[:], ti[:])
                nc.vector.tensor_tensor(tmp1[:], fdy[:], dy, ALU.is_gt)
                nc.vector.tensor_tensor(fdy[:], fdy[:], tmp1[:], ALU.subtract)
                # floor(dx)
                nc.vector.tensor_copy(ti[:], dx)
                nc.vector.tensor_copy(fdx[:], ti[:])
                nc.vector.tensor_tensor(tmp1[:], fdx[:], dx, ALU.is_gt)
                nc.vector.tensor_tensor(fdx[:], fdx[:], tmp1[:], ALU.subtract)

                ly, lx = T('ly'), T('lx')
                nc.vector.tensor_tensor(ly[:], dy, fdy[:], ALU.subtract)
                nc.vector.tensor_tensor(lx[:], dx, fdx[:], ALU.subtract)

                y0, x0 = T('y0'), T('x0')
                nc.vector.tensor_tensor(y0[:], hgk[:], fdy[:], ALU.add)
                nc.vector.tensor_tensor(x0[:], wgk[:], fdx[:], ALU.add)

                # y anchor + tap-swap terms
                ay, y01, ayp = T('ay'), T('y01'), T('ayp')
                f0, f1, f3 = T('f0'), T('f1'), T('f3')
                nc.vector.tensor_scalar(ay[:], y0[:], 62.0, 0.0, ALU.min, ALU.max)
                nc.vector.tensor_scalar(y01[:], y0[:], 1.0, None, ALU.add)
                nc.vector.tensor_scalar(ayp[:], ay[:], 1.0, None, ALU.add)
                nc.vector.tensor_tensor(f0[:], y0[:], ay[:], ALU.is_equal)
                nc.vector.tensor_tensor(f1[:], y01[:], ay[:], ALU.is_equal)
                nc.vector.tensor_tensor(f3[:], y0[:], ayp[:], ALU.is_equal)

                # x anchor + tap-swap terms
                bx, x01, bxp = T('bx'), T('x01'), T('bxp')
                e0, e1, e3 = T('e0'), T('e1'), T('e3')
                nc.vector.tensor_scalar(bx[:], x0[:], 62.0, 0.0, ALU.min, ALU.max)
                nc.vector.tensor_scalar(x01[:], x0[:], 1.0, None, ALU.add)
                nc.vector.tensor_scalar(bxp[:], bx[:], 1.0, None, ALU.add)
                nc.vector.tensor_tensor(e0[:], x0[:], bx[:], ALU.is_equal)
                nc.vector.tensor_tensor(e1[:], x01[:], bx[:], ALU.is_equal)
                nc.vector.tensor_tensor(e3[:], x0[:], bxp[:], ALU.is_equal)

                # wyA = (1-ly)f0 + ly*f1 ; wyB = ly*f0 + (1-ly)*f3
                wyA, wyB = T('wyA'), T('wyB')
                nc.vector.tensor_tensor(tmp1[:], ly[:], f0[:], ALU.mult)
                nc.vector.tensor_tensor(wyA[:], f0[:], tmp1[:], ALU.subtract)
                nc.vector.tensor_tensor(tmp2[:], ly[:], f1[:], ALU.mult)
                nc.vector.tensor_tensor(wyA[:], wyA[:], tmp2[:], ALU.add)
                nc.vector.tensor_tensor(tmp2[:], ly[:], f3[:], ALU.mult)
                nc.vector.tensor_tensor(tmp3[:], f3[:], tmp2[:], ALU.subtract)
                nc.vector.tensor_tensor(wyB[:], tmp1[:], tmp3[:], ALU.add)

                wxA, wxB = T('wxA'), T('wxB')
                nc.vector.tensor_tensor(tmp1[:], lx[:], e0[:], ALU.mult)
                nc.vector.tensor_tensor(wxA[:], e0[:], tmp1[:], ALU.subtract)
                nc.vector.tensor_tensor(tmp2[:], lx[:], e1[:], ALU.mult)
                nc.vector.tensor_tensor(wxA[:], wxA[:], tmp2[:], ALU.add)
                nc.vector.tensor_tensor(tmp2[:], lx[:], e3[:], ALU.mult)
                nc.vector.tensor_tensor(tmp3[:], e3[:], tmp2[:], ALU.subtract)
                nc.vector.tensor_tensor(wxB[:], tmp1[:], tmp3[:], ALU.add)

                # tap weights (Z block order: (yA,xA),(yB,xA),(yA,xB),(yB,xB))
                nc.vector.tensor_tensor(w4[:, :, :, 0], wyA[:], wxA[:], ALU.mult)
                nc.vector.tensor_tensor(w4[:, :, :, 1], wyB[:], wxA[:], ALU.mult)
                nc.vector.tensor_tensor(w4[:, :, :, 2], wyA[:], wxB[:], ALU.mult)
                nc.vector.tensor_tensor(w4[:, :, :, 3], wyB[:], wxB[:], ALU.mult)

                # gather index idx = ay*64 + bx  (exact int in f32, <= 4030)
                idxf = T('idxf')
                nc.vector.tensor_scalar(tmp1[:], ay[:], 64.0, None, ALU.mult)
                nc.vector.tensor_tensor(idxf[:], tmp1[:], bx[:], ALU.add)

                # ---------- on-chip idx wrap (PE transposes) ----------
                # Round 1: pixel-major [jp, jt] -> j-natural rows [jt, jp].
                idxT = apool.tile([NJT, NK, 128], F32, tag='idxT')
                for k in range(NK):
                    ps = ps_small.tile([NJT, 128], F32, tag='ps')
                    nc.tensor.transpose(ps[:], idxf[:, :, k], ident[:])
                    nc.vector.tensor_copy(idxT[:, k, :], ps[:])
                # Round 2: [jt, fb*16+p] -> [p, (q, fa=jtl)] per (k, fb),
                # cast to i16 on the PSUM evacuation.
                for k in range(NK):
                    for fb in range(8):
                        ps = ps_small.tile([16, NJT], F32, tag='ps16',
                                           name='ps16')
                        nc.tensor.transpose(
                            ps[:], idxT[:, k, 16 * fb:16 * (fb + 1)],
                            ident[:NJT, :NJT])
                        nc.vector.tensor_copy(
                            idxwr[0:16, k, :, :, fb],
                            ps[:].rearrange('p (h a) -> p h a', h=NHALF))
                # replicate 16 -> 128 partitions (3 doubling copies)
                for g in (16, 32, 64):
                    nc.sync.dma_start(idxwr[g:2 * g], idxwr[0:g])

            # ======== phase 2: gather + weighting + transpose + conv ========
            if phase != 1:
                _phase2(nc, tc, wdefT, bdef, idxwr, w4, z_d, y_d, phase)
            if phase != 3:
                with tc.tile_pool(name='zz', bufs=1) as zp:
                    zt = zp.tile([128, HW], F32, tag='zt')
                    nc.vector.memset(zt[:], 0)
                    for oh in range(2):
                        nc.sync.dma_start(
                            y_d.ap()[128 * oh:128 * (oh + 1), :], zt[:])

    nc.finalize()
    _split_multiwaits(nc)
    return nc


def _phase2(nc, tc, wdefT, bdef, idxwr, w4, z_d, y_d, phase=3):
    # Tap weighting: ScalarE produces the last tap's product (per-partition
    # scale, own SBUF ports); DVE runs three scalar_tensor_tensor ops, the
    # first absorbing the ACT product as its in1.  stt overlaps SWDGE gather
    # descriptor generation nearly cleanly (measured), unlike tensor_scalar
    # whose 2-port perf mode serializes against the Q7 shared-port lock.
    with (
        tc.tile_pool(name='gath', bufs=4) as gpool,
        tc.tile_pool(name='tmpa', bufs=8) as tpool,
        tc.tile_pool(name='val', bufs=3) as vpool,
        tc.tile_pool(name='valT', bufs=3) as vtpool,
        tc.tile_pool(name='outp', bufs=2) as opool,
        tc.tile_pool(name='pcv', bufs=1, space='PSUM') as pcv,
    ):
        for half in range(NHALF):
            if phase == 3:
                psc = [pcv.tile([128, 512], F32, tag=f'psc{i}',
                                name=f'psc{i}') for i in range(8)]
            for k in range(NK):
                # ---- gather: one 1KB descriptor per pixel, all 4 taps ----
                G = gpool.tile([128, JH // 128, ZW], BF16, tag='G', name='G')
                in_ap = bass.AP(
                    tensor=z_d, offset=0,
                    ap=[[ZW, HW], [1, ZW]],
                )
                nc.gpsimd.dma_gather(
                    out_ap=G[:],
                    in_ap=in_ap,
                    idxs_ap=idxwr[:, k, half, :, :],
                    num_idxs=JH,
                    num_idxs_reg=JH,
                    elem_size=ZW,
                    elem_step=ZW,
                    transpose=False,
                    single_packet=False,
                )
                if phase == 21:
                    continue

                # ---- tap weighting (pixel-major, per-partition scalars) ----
                val = vpool.tile([128, JH // 128, CIN], BF16, tag='val',
                                 name='val')
                for jtl in range(JH // 128):
                    jt = half * (JH // 128) + jtl
                    vsl = val[:, jtl, :]
                    tmpa = tpool.tile([128, CIN], BF16, tag='tmpa',
                                      name='tmpa')
                    nc.scalar.mul(tmpa[:], G[:, jtl, 3 * CIN:4 * CIN],
                                  w4[:, jt, k, 3].unsqueeze(1))
                    nc.vector.scalar_tensor_tensor(
                        vsl, G[:, jtl, 0:CIN],
                        w4[:, jt, k, 0].unsqueeze(1), tmpa[:],
                        ALU.mult, ALU.add)
                    for t in (1, 2):
                        nc.vector.scalar_tensor_tensor(
                            vsl, G[:, jtl, t * CIN:(t + 1) * CIN],
                            w4[:, jt, k, t].unsqueeze(1), vsl,
                            ALU.mult, ALU.add)

                # ---- batched xbar transpose -> channel-major j-natural ----
                valT = vtpool.tile([128, JH // 128, 128], BF16, tag='valT',
                                   name='valT')
                nc.sync.dma_start_transpose(valT[:], val[:])

                if phase != 3:
                    continue
                # ---- main conv: accumulate over k in PSUM ----
                for oh in range(2):
                    for jc in range(4):
                        nc.tensor.matmul(
                            psc[oh * 4 + jc][:],
                            lhsT=wdefT[:, k, 128 * oh:128 * (oh + 1)],
                            rhs=valT[:, 4 * jc:4 * (jc + 1), :],
                            start=(k == 0), stop=(k == NK - 1))

            if phase != 3:
                continue
            for oh in range(2):
                for jc in range(4):
                    yo = opool.tile([128, 512], F32, tag='yo')
                    nc.scalar.activation(
                        yo[:], psc[oh * 4 + jc][:], ACTF.Relu,
                        bias=bdef[:, oh:oh + 1], scale=1.0)
                    j0 = half * JH + 512 * jc
                    nc.sync.dma_start(
                        y_d.ap()[128 * oh:128 * (oh + 1), j0:j0 + 512],
                        yo[:])


def _host_prep(x, w_off, b_off, w_def, b_def):
    """Build per-core input maps."""
    x = np.asarray(x, np.float32)
    w_off = np.asarray(w_off, np.float32)
    b_off = np.asarray(b_off, np.float32)
    w_def = np.asarray(w_def, np.float32)
    b_def = np.asarray(b_def, np.float32)

    woffT = np.stack([w_off[:, :, s // 3, s % 3].T for s in range(9)])
    woffT = _to_bf16(np.ascontiguousarray(woffT, np.float32))  # [9, 128, 18]
    wdefT = np.stack([w_def[:, :, s // 3, s % 3].T for s in range(9)])
    wdefT = _to_bf16(np.ascontiguousarray(wdefT))             # [9, 128, 256]
    bdef2 = np.ascontiguousarray(b_def.reshape(2, 128).T)     # [128, 2]
    ident = np.eye(128, dtype=np.float32)

    jp = np.arange(128)[:, None, None]
    jt = np.arange(NJT)[None, :, None]
    kk = np.arange(NK)[None, None, :]
    j = jt * 128 + jp
    ky = np.array(KY, np.float32)[kk]
    kx = np.array(KX, np.float32)[kk]
    hgk = (j // 64).astype(np.float32) + ky
    wgk = (j % 64).astype(np.float32) + kx
    hgk = np.ascontiguousarray(np.broadcast_to(hgk, (128, NJT, NK)), np.float32)
    wgk = np.ascontiguousarray(np.broadcast_to(wgk, (128, NJT, NK)), np.float32)

    xp = np.pad(x, ((0, 0), (0, 0), (1, 1), (1, 1))).reshape(B, CIN, 66 * 66)
    xpb = _to_bf16(xp)

    shared = {
        'woffT': woffT,
        'boff': np.ascontiguousarray(b_off.reshape(18, 1)),
        'wdefT': wdefT,
        'bdef': bdef2,
        'ident': ident,
        'hgk': hgk,
        'wgk': wgk,
    }
    in_maps = []
    for b in range(B):
        m = dict(shared)
        m['xp'] = np.ascontiguousarray(xpb[b])
        in_maps.append(m)
    return in_maps


def _to_bf16(a):
    import ml_dtypes
    return a.astype(ml_dtypes.bfloat16)


LAST_RESULTS = None


def _ensure_trace_support():
    """Register the NTFF profile hook that the slim agent image lacks, and
    stub out the artifact upload. Only used when KBENCH_TRACE is set."""
    import contextlib
    import ctypes
    import types

    import concourse.bass_utils as bu
    bu.upload_artifacts = lambda tmpdir: tmpdir

    if 'antenv.axon_hooks' in sys.modules:
        return
    so_path = '/opt/axon/libaxon_pjrt.so'
    if not os.path.exists(so_path):
        return
    lib = ctypes.CDLL(so_path)
    if not hasattr(lib, 'axon_start_nrt_profile'):
        return
    lib.axon_start_nrt_profile.argtypes = [
        ctypes.POINTER(ctypes.c_int64), ctypes.c_size_t]
    lib.axon_start_nrt_profile.restype = ctypes.c_int64
    lib.axon_stop_nrt_profile.argtypes = [ctypes.c_char_p]
    lib.axon_stop_nrt_profile.restype = ctypes.c_int64

    @contextlib.contextmanager
    def _hook(output_dir, device_ids):
        import jax
        jax.devices()
        if device_ids:
            ids = (ctypes.c_int64 * len(device_ids))(*device_ids)
            rc = lib.axon_start_nrt_profile(ids, len(device_ids))
        else:
            rc = lib.axon_start_nrt_profile(None, 0)
        if rc != 0:
            raise RuntimeError(f'axon_start_nrt_profile rc={rc}')
        try:
            yield
        finally:
            n = lib.axon_stop_nrt_profile(str(output_dir).encode())
            print(f'profile: {n} file(s) written to {output_dir}',
                  file=sys.stderr)

    mod = types.ModuleType('antenv.axon_hooks')
    mod.get_axon_ntff_profile_hook = lambda: _hook
    mod.set_axon_ntff_profile_hook = lambda h: None
    sys.modules['antenv.axon_hooks'] = mod


def kernel(x, w_off, b_off, w_def, b_def):
    global LAST_RESULTS
    if 'nc' not in _CACHE:
        _CACHE['nc'] = _build_program(
            phase=int(os.environ.get('KBENCH_PHASE', '3')))
    nc = _CACHE['nc']
    in_maps = _host_prep(x, w_off, b_off, w_def, b_def)
    trace = bool(os.environ.get('KBENCH_TRACE'))
    if trace:
        _ensure_trace_support()
    res = run_bass_kernel_spmd(
        nc, in_maps, core_ids=list(range(B)),
        trace=trace,
    )
    LAST_RESULTS = res
    out = np.stack([res.results[b]['y'].reshape(COUT, H, W) for b in range(B)])
    return out.astype(np.float32)
